# revision 1
# baseline (speedup 1.0000x reference)
"""Trainium2 Bass kernel for nn_BezierRenderer.

Renders B=16 gaussian-window "bezier" strokes onto 512x512 canvases:
  out[b] = max over 10 segments of clip((thick_b - dist(pixel, seg)) / thick_b, 0, 1)

Strategy
--------
The active area (pixels within `thick` of any segment) is ~1.6% of the dense
B*P*H*W domain, so the kernel rasterizes only per-segment bounding-box
windows.  For each segment two planes that are affine in pixel coordinates
are produced by one small fp32 PE matmul (K=2, stationary = [1; coord]):

   z  = projection coordinate / sqrt(d2+1e-5), shifted by -m/2
   w  = exact perpendicular offset  (w^2 = |p-v|^2 - s^2/d2)

With m = sqrt(d2+1e-5), the reference's clamped-projection distance is
   dist^2 = w^2 + relu(|z| - m/2)^2        (to within ~1e-10 absolute)
The per-slot pipeline is:  PE matmul -> ACT Abs + ACT Square (PSUM->SBUF)
-> DVE tensor_scalar relu (2x mode) -> DVE square -> DVE add -> DVE min
into a per-stroke-tile running-min plane.  Epilogue per tile:
ACT sqrt(minacc * invthick^2), ACT relu(1 - dist/thick), DMA out.

Work is split across the 8 NeuronCores stroke-wise (greedy balance by
estimated DVE cycles).  Each core gets its own specialized Bass program
(the sparse windows are baked in at trace time); programs run concurrently,
one per core, via PJRT with jax.default_device pinning.  The host only
mirrors the reference's tiny stroke-endpoint setup (160 control points),
packs the per-slot affine coefficients, and scatters the computed tiles
into the zero canvas.
"""

import threading
from contextlib import ExitStack

import numpy as np

# ---------------------------------------------------------------------------
# problem constants (hardcoded per contract; kernel.py must be self-contained)
# ---------------------------------------------------------------------------
SIZE = 512
NUM_CTRL = 4
P = 10  # samples per curve -> 10 segments
B = 16
N_CORES = 8
MARGIN_PAD = 1.5  # extra pixels beyond thick for bbox safety
MAX_W = 256  # max window width per matmul (PSUM bank = 512 fp32 = 2w)
INIT_MIN = 1.0e12


# ---------------------------------------------------------------------------
# host-side geometry (mirrors reference.py numerics)
# ---------------------------------------------------------------------------
def _bezier_weights():
    M = 2 * P
    n = np.arange(M) - (M - 1) / 2.0
    gaus = np.exp(-0.5 * (n / 2.0) ** 2) * 0.75
    W = np.zeros((NUM_CTRL, P), dtype=np.float32)
    for i in range(NUM_CTRL):
        start = int(P - P * (i / (NUM_CTRL - 1)))
        W[i, :] = gaus[start : start + P]
    return W


def _host_strokes(trajectories, thicknesses):
    W = _bezier_weights()
    traj = np.asarray(trajectories, dtype=np.float32)
    sample = np.einsum("bck,kp->bpc", traj, W).astype(np.float32)
    last = traj[:, :, 3][:, None, :]
    stroke = np.concatenate([sample, last], axis=1).astype(np.float32)
    stroke = stroke * np.float32(SIZE)  # (B, P+1, 2) [y, x]
    vs = stroke[:, :-1]  # (B, P, 2)
    ws = stroke[:, 1:]
    th = np.asarray(thicknesses, dtype=np.float32)[:, 0] * np.float32(2.0) + np.float32(0.5)
    thick = np.float32(2.0) * th.sum(-1, dtype=np.float32)  # (B,)
    return vs, ws, thick


# ---------------------------------------------------------------------------
# work planning: per stroke -> orientation, part-blocks, per-(seg,block) slots
# ---------------------------------------------------------------------------
class Slot:
    __slots__ = ("tile_idx", "f_lo", "f_w", "row0_z", "row1_z", "row0_w", "row1_w", "h")

    def __init__(self, tile_idx, f_lo, f_w, row0_z, row1_z, row0_w, row1_w, h):
        self.tile_idx = tile_idx
        self.f_lo = f_lo
        self.f_w = f_w
        self.row0_z = row0_z  # (f_w,) float32 - const row of z-plane (centered clamp)
        self.row1_z = row1_z  # scalar - coefficient on p_hat
        self.row0_w = row0_w
        self.row1_w = row1_w
        self.h = h  # half-width of clamp (m/2)


class Tile:
    __slots__ = ("stroke", "transposed", "p_lo", "p_ext", "f_lo", "f_ext", "invth2", "thick")

    def __init__(self, stroke, transposed, p_lo, p_ext, f_lo, f_ext, invth2, thick):
        self.stroke = stroke
        self.transposed = transposed  # True: partition axis = x, free axis = y
        self.p_lo = p_lo
        self.p_ext = p_ext
        self.f_lo = f_lo
        self.f_ext = f_ext
        self.invth2 = invth2
        self.thick = thick


def _plan_stroke(b, vs, ws, thick):
    """Returns (tiles, slots) for one stroke, or ([], []) if fully off-canvas."""
    v = vs[b].astype(np.float64)  # (P, 2) [y, x]
    w = ws[b].astype(np.float64)
    margin = float(thick[b]) + MARGIN_PAD

    lo = np.minimum(v, w).min(axis=0) - margin  # (2,)
    hi = np.maximum(v, w).max(axis=0) + margin
    ylo = max(0, int(np.floor(lo[0])))
    yhi = min(SIZE, int(np.ceil(hi[0])) + 1)
    xlo = max(0, int(np.floor(lo[1])))
    xhi = min(SIZE, int(np.ceil(hi[1])) + 1)
    if yhi <= ylo or xhi <= xlo:
        return [], []

    yext, xext = yhi - ylo, xhi - xlo
    # partition axis = smaller extent (fewer 128-blocks)
    transposed = xext < yext
    if transposed:
        p_lo0, p_ext_tot, f_lo0, f_hi0 = xlo, xext, ylo, yhi
        PAX, FAX = 1, 0
    else:
        p_lo0, p_ext_tot, f_lo0, f_hi0 = ylo, yext, xlo, xhi
        PAX, FAX = 0, 1

    invth = 1.0 / float(thick[b])
    tiles = []
    slots = []
    n_pb = (p_ext_tot + 127) // 128
    for pb in range(n_pb):
        p_lo = p_lo0 + pb * 128
        p_ext = min(128, p_lo0 + p_ext_tot - p_lo)
        # tile free extent = stroke free extent (shared across segments), even
        f_lo = f_lo0 & ~1
        f_ext = min(SIZE - f_lo, (f_hi0 - f_lo + 1) & ~1)
        tile = Tile(
            b, transposed, p_lo, p_ext, f_lo, f_ext,
            np.float32(invth * invth), np.float32(thick[b]),
        )
        tile_idx = None  # assigned by caller
        tiles.append(tile)

        p_c = p_lo + (p_ext - 1) / 2.0  # center of the partition block

        for s in range(P):
            vy, vx = v[s]
            wy, wx = w[s]
            dy, dx = wy - vy, wx - vx
            d2 = dy * dy + dx * dx
            d2p = d2 + 1e-5
            m = np.sqrt(d2p)
            h = m / 2.0
            inv_sd2p = 1.0 / m
            inv_sd2 = 1.0 / np.sqrt(d2) if d2 > 1e-4 else None

            # window on free axis: clip segment to this part-block's range
            vp, vf = (vy, vx) if not transposed else (vx, vy)
            wp, wf = (wy, wx) if not transposed else (wx, wy)
            blo, bhi = p_lo - margin, p_lo + p_ext - 1 + margin
            # param range of segment within [blo, bhi] on the partition axis
            if abs(wp - vp) < 1e-12:
                t0, t1 = 0.0, 1.0
                if vp < blo or vp > bhi:
                    continue
            else:
                ta = (blo - vp) / (wp - vp)
                tb = (bhi - vp) / (wp - vp)
                t0, t1 = max(0.0, min(ta, tb)), min(1.0, max(ta, tb))
                if t1 < t0:
                    continue
            fa = vf + t0 * (wf - vf)
            fb = vf + t1 * (wf - vf)
            w_lo = max(f_lo, int(np.floor(min(fa, fb) - margin)) & ~1)
            w_hi = min(f_lo + f_ext, (int(np.ceil(max(fa, fb) + margin)) + 2) & ~1)
            if w_hi <= w_lo:
                continue

            # affine coefficients: z and w planes over (p_hat, f)
            # dp/df = diff along partition/free axes
            dp, df = (dy, dx) if not transposed else (dx, dy)
            f = np.arange(w_lo, w_hi, dtype=np.float64)
            # s_dot(p, f) = (p - vp)*dp + (f - vf)*df ; p = p_c + p_hat
            # z = s_dot / m - h  (shifted so clamp window is [-h, h])
            if inv_sd2 is not None:
                row0_z = ((p_c - vp) * dp + (f - vf) * df) * inv_sd2p - h
                row1_z = dp * inv_sd2p
                # w_perp = ((p - vp)*df - (f - vf)*dp) / sqrt(d2)   (exact geometry)
                row0_w = ((p_c - vp) * df - (f - vf) * dp) * inv_sd2
                row1_w = df * inv_sd2
                h_clamp = h
            else:
                # degenerate (tiny) segment: point distance to v
                row0_z = (p_c - vp) + 0.0 * f
                row1_z = 1.0
                row0_w = f - vf
                row1_w = 0.0
                h_clamp = 0.0

            # split long windows into chunks <= MAX_W
            wdt = w_hi - w_lo
            n_ch = (wdt + MAX_W - 1) // MAX_W
            ch = ((wdt + n_ch - 1) // n_ch + 1) & ~1
            off = 0
            while off < wdt:
                cw = min(ch, wdt - off)
                slots.append(
                    Slot(
                        len(tiles) - 1,  # local tile index within this stroke
                        w_lo + off - f_lo,
                        cw,
                        row0_z[off : off + cw].astype(np.float32),
                        np.float32(row1_z),
                        row0_w[off : off + cw].astype(np.float32),
                        np.float32(row1_w),
                        np.float32(h_clamp),
                    )
                )
                off += cw
    return tiles, slots


def _plan_all(vs, ws, thick):
    """Plan tiles/slots for every stroke and greedily balance across cores.
    The unit of distribution is one TILE (pixel-disjoint by construction,
    so no cross-core combining is ever needed)."""
    units = []  # (cost, tile, its_slots)
    for b in range(B):
        tiles, slots = _plan_stroke(b, vs, ws, thick)
        for t_i, t in enumerate(tiles):
            ts_slots = [s for s in slots if s.tile_idx == t_i]
            cost = sum(232 + 3.5 * s.f_w for s in ts_slots) + 800
            units.append((cost, t, ts_slots))
    units.sort(key=lambda u: u[0], reverse=True)
    core_cost = [0.0] * N_CORES
    core_work = [[] for _ in range(N_CORES)]  # list of (tiles, slots) groups
    for cost, t, ts_slots in units:
        c = min(range(N_CORES), key=lambda i: core_cost[i])
        core_cost[c] += cost
        for s in ts_slots:
            s.tile_idx = 0  # single-tile group
        core_work[c].append(([t], ts_slots))
    return core_work


# ---------------------------------------------------------------------------
# bass program construction (one per core)
# ---------------------------------------------------------------------------
def _split_multiwait(nc, mybir):
    """This container's walrus accepts at most ONE semaphore wait per
    instruction; Tile attaches several.  Split extras onto NoOps."""
    for fn in nc.m.functions:
        for bb in fn.blocks:
            insts = bb.instructions
            idx = 0
            while idx < len(insts):
                inst = insts[idx]
                si = inst.sync_info
                ow = list(si.on_wait) if (si and si.on_wait) else []
                if len(ow) > 1:
                    si.on_wait = ow[-1:]
                    for j, w in enumerate(ow[:-1]):
                        nop = mybir.InstNoOp(
                            name=f"{inst.name}-ws{j}",
                            engine=inst.engine,
                            ins=[],
                            outs=[],
                            sync_info=mybir.SyncInfo(on_wait=[w], on_update=[]),
                        )
                        nc.register_instruction(nop, overwrite=True)
                        insts.insert(idx, nop)
                        idx += 1
                idx += 1


def _build_core_program(work, repeat=1):
    """work: list of (tiles, slots) per stroke.  Returns (nc, in_map, meta).
    repeat>1 re-traces the rasterization loop (for differential timing)."""
    import concourse.bass as bass
    import concourse.mybir as mybir
    import concourse.tile as tile_mod

    # flatten tiles; remap slot tile indices
    all_tiles = []
    all_slots = []
    for tiles, slots in work:
        base = len(all_tiles)
        all_tiles.extend(tiles)
        for s in slots:
            all_slots.append((base + s.tile_idx, s))
    n_tiles = max(1, len(all_tiles))

    # ---- pack host arrays ----
    # RHS [2, total_cols]: per slot 2*f_w cols: [z-half | w-half]
    # row0 = const row, row1 = p_hat coefficient (replicated)
    offs = []
    cols = 0
    for _, s in all_slots:
        offs.append(cols)
        cols += 2 * s.f_w
    cols = max(2, cols)
    rhs = np.zeros((2, cols), dtype=np.float32)
    for (ti, s), off in zip(all_slots, offs):
        fw = s.f_w
        rhs[0, off : off + fw] = s.row0_z
        rhs[1, off : off + fw] = s.row1_z
        rhs[0, off + fw : off + 2 * fw] = s.row0_w
        rhs[1, off + fw : off + 2 * fw] = s.row1_w

    # STAT [2, 128 * n_tiles]: per tile stationary [ones; p_hat]
    stat = np.zeros((2, 128 * n_tiles), dtype=np.float32)
    for t_i, t in enumerate(all_tiles):
        p_c = t.p_lo + (t.p_ext - 1) / 2.0
        stat[0, t_i * 128 : t_i * 128 + t.p_ext] = 1.0
        stat[1, t_i * 128 : t_i * 128 + t.p_ext] = (
            np.arange(t.p_lo, t.p_lo + t.p_ext, dtype=np.float64) - p_c
        ).astype(np.float32)

    # SCAL [128, n_slots + 2*n_tiles]: bcast scalars: per-slot h; per-tile invth2
    nscal = max(1, len(all_slots) + n_tiles)
    scal = np.zeros((128, nscal), dtype=np.float32)
    for i, (_, s) in enumerate(all_slots):
        scal[:, i] = s.h
    for t_i, t in enumerate(all_tiles):
        scal[:, len(all_slots) + t_i] = t.invth2

    # ---- trace program ----
    nc = bass.Bass()
    rhs_ext = nc.dram_tensor("rhs", list(rhs.shape), mybir.dt.float32, kind="ExternalInput")
    stat_ext = nc.dram_tensor("stat", list(stat.shape), mybir.dt.float32, kind="ExternalInput")
    scal_ext = nc.dram_tensor("scal", list(scal.shape), mybir.dt.float32, kind="ExternalInput")
    out_ext = nc.dram_tensor(
        "out", [n_tiles, 128, SIZE], mybir.dt.float32, kind="ExternalOutput"
    )

    with tile_mod.TileContext(nc) as tc:
        with ExitStack() as ctx:
            const_pool = ctx.enter_context(tc.tile_pool(name="const", bufs=1))
            minacc_pool = ctx.enter_context(tc.tile_pool(name="minacc", bufs=1))
            sb = ctx.enter_context(tc.tile_pool(name="work", bufs=10))
            psum = ctx.enter_context(tc.tile_pool(name="psum", bufs=6, space="PSUM"))
            outp = ctx.enter_context(tc.tile_pool(name="outp", bufs=2))

            t_rhs = const_pool.tile(list(rhs.shape), mybir.dt.float32)
            nc.gpsimd.dma_start(t_rhs[:], rhs_ext[:])
            t_stat = const_pool.tile(list(stat.shape), mybir.dt.float32)
            nc.gpsimd.dma_start(t_stat[:], stat_ext[:])
            t_scal = const_pool.tile(list(scal.shape), mybir.dt.float32)
            nc.gpsimd.dma_start(t_scal[:], scal_ext[:])

            for _rep in range(repeat):
              # persistent min-accumulator per tile
              t_min = []
              for t_i, t in enumerate(all_tiles):
                  m = minacc_pool.tile([128, t.f_ext], mybir.dt.float32, tag=f"min{t_i}")
                  nc.gpsimd.memset(m[:], INIT_MIN)
                  t_min.append(m)

              # main sparse rasterization loop
              for i_slot, ((ti, s), off) in enumerate(zip(all_slots, offs)):
                  t = all_tiles[ti]
                  fw = s.f_w
                  pe = t.p_ext
                  zp = psum.tile([128, 2 * fw], mybir.dt.float32, tag="zp")
                  nc.tensor.matmul(
                      zp[:pe, :],
                      t_stat[:, ti * 128 : ti * 128 + pe],
                      t_rhs[:, off : off + 2 * fw],
                      start=True,
                      stop=True,
                  )
                  a = sb.tile([128, fw], mybir.dt.float32, tag="a")
                  nc.scalar.activation(a[:pe, :], zp[:pe, :fw], mybir.ActivationFunctionType.Abs)
                  w2 = sb.tile([128, fw], mybir.dt.float32, tag="w2")
                  nc.scalar.activation(
                      w2[:pe, :], zp[:pe, fw : 2 * fw], mybir.ActivationFunctionType.Square
                  )
                  e = sb.tile([128, fw], mybir.dt.float32, tag="e")
                  nc.vector.tensor_scalar(
                      e[:pe, :], a[:pe, :], t_scal[:pe, i_slot : i_slot + 1], 0.0,
                      mybir.AluOpType.subtract, mybir.AluOpType.max,
                  )
                  e2 = sb.tile([128, fw], mybir.dt.float32, tag="e2")
                  nc.vector.scalar_tensor_tensor(
                      e2[:pe, :], e[:pe, :], 1.0, e[:pe, :],
                      mybir.AluOpType.mult, mybir.AluOpType.mult,
                  )
                  d = sb.tile([128, fw], mybir.dt.float32, tag="d")
                  nc.vector.tensor_tensor(d[:pe, :], e2[:pe, :], w2[:pe, :], mybir.AluOpType.add)
                  msl = t_min[ti][:pe, s.f_lo : s.f_lo + fw]
                  nc.vector.tensor_tensor(msl, msl, d[:pe, :], mybir.AluOpType.min)

              # epilogue per tile: dark = relu(1 - sqrt(minacc)/thick)
              for t_i, t in enumerate(all_tiles):
                  pe = t.p_ext
                  fe = t.f_ext
                  sq = outp.tile([128, SIZE], mybir.dt.float32, tag="sq")
                  nc.scalar.activation(
                      sq[:pe, :fe], t_min[t_i][:pe, :],
                      mybir.ActivationFunctionType.Sqrt,
                      scale=t_scal[:pe, len(all_slots) + t_i : len(all_slots) + t_i + 1],
                  )
                  dk = outp.tile([128, SIZE], mybir.dt.float32, tag="dk")
                  nc.scalar.activation(
                      dk[:pe, :fe], sq[:pe, :fe],
                      mybir.ActivationFunctionType.Relu, bias=1.0, scale=-1.0,
                  )
                  nc.gpsimd.dma_start(out_ext[t_i, :pe, :fe], dk[:pe, :fe])

    _split_multiwait(nc, mybir)
    in_map = {"rhs": rhs, "stat": stat, "scal": scal}
    meta = all_tiles
    return nc, in_map, meta


# ---------------------------------------------------------------------------
# MPMD runner: one program per core, pinned via jax.default_device
# ---------------------------------------------------------------------------
def _make_exec(nc, in_map, device):
    """Build a cached jitted executor for one core's program. Returns run()
    -> dict of output arrays."""
    import jax
    import concourse.mybir as mybir
    from concourse import bass2jax

    bass2jax.install_neuronx_cc_hook()
    partition_name = nc.partition_id_tensor.name if nc.partition_id_tensor else None
    in_names, out_names, out_avals, zero_shapes = [], [], [], []
    for alloc in nc.m.functions[0].allocations:
        if not isinstance(alloc, mybir.MemoryLocationSet):
            continue
        name = alloc.memorylocations[0].name
        if alloc.kind == "ExternalInput":
            if name != partition_name:
                in_names.append(name)
        elif alloc.kind == "ExternalOutput":
            out_names.append(name)
            shape = tuple(alloc.tensor_shape)
            dtype = mybir.dt.np(alloc.dtype)
            out_avals.append(jax.core.ShapedArray(shape, dtype))
            zero_shapes.append((shape, dtype))
    n_params = len(in_names)
    all_in_names = list(in_names) + out_names
    if partition_name is not None:
        all_in_names.append(partition_name)
    donate = tuple(range(n_params, n_params + len(out_names)))

    def _body(*args):
        operands = list(args)
        if partition_name is not None:
            operands.append(bass2jax.partition_id_tensor())
        outs = bass2jax._bass_exec_p.bind(
            *operands,
            out_avals=tuple(out_avals),
            in_names=tuple(all_in_names),
            out_names=tuple(out_names),
            lowering_input_output_aliases=(),
            sim_require_finite=True,
            sim_require_nnan=True,
            nc=nc,
        )
        return tuple(outs)

    fn = jax.jit(_body, donate_argnums=donate, keep_unused=True)
    args = [np.asarray(in_map[n]) for n in in_names]

    def run(block=True):
        with jax.default_device(device):
            outs = fn(*args, *[np.zeros(s, d) for s, d in zero_shapes])
        if block:
            for o in outs:
                o.block_until_ready()
        return {name: outs[i] for i, name in enumerate(out_names)}

    return run


_CACHE = {}


def _prepare(trajectories, thicknesses):
    import jax

    key = (np.asarray(trajectories).tobytes(), np.asarray(thicknesses).tobytes())
    if key in _CACHE:
        return _CACHE[key]
    vs, ws, thick = _host_strokes(trajectories, thicknesses)
    core_work = _plan_all(vs, ws, thick)
    progs = [_build_core_program(core_work[c]) for c in range(N_CORES)]
    devices = jax.devices()[:N_CORES]
    runners = [None] * N_CORES
    errors = []

    def make(c):
        try:
            nc, in_map, _ = progs[c]
            runners[c] = _make_exec(nc, in_map, devices[c])
            runners[c]()  # warm up: compile + first exec
        except Exception as e:  # pragma: no cover
            errors.append((c, e))

    threads = [threading.Thread(target=make, args=(c,)) for c in range(N_CORES)]
    for t in threads:
        t.start()
    for t in threads:
        t.join()
    if errors:
        raise errors[0][1]
    _CACHE[key] = (progs, runners)
    return _CACHE[key]


def kernel(trajectories, thicknesses):
    trajectories = np.asarray(trajectories)
    thicknesses = np.asarray(thicknesses)
    progs, runners = _prepare(trajectories, thicknesses)

    results = [None] * N_CORES
    errors = []

    def runner(c):
        try:
            results[c] = runners[c]()
        except Exception as e:  # pragma: no cover
            errors.append((c, e))

    threads = [threading.Thread(target=runner, args=(c,)) for c in range(N_CORES)]
    for t in threads:
        t.start()
    for t in threads:
        t.join()
    if errors:
        raise errors[0][1]

    # assemble full output on host
    canvas = np.zeros((B, SIZE, SIZE), dtype=np.float32)
    for c in range(N_CORES):
        _, _, tiles = progs[c]
        out = np.asarray(results[c]["out"])
        for t_i, t in enumerate(tiles):
            block = out[t_i, : t.p_ext, : t.f_ext]
            if t.transposed:
                canvas[t.stroke, t.f_lo : t.f_lo + t.f_ext, t.p_lo : t.p_lo + t.p_ext] = block.T
            else:
                canvas[t.stroke, t.p_lo : t.p_lo + t.p_ext, t.f_lo : t.f_lo + t.f_ext] = block
    return canvas


def time_cores(inputs, repeats=1000, r_hi=9, rounds=4, cores=None):
    """Per-core device-kernel time via differential replication:
    the program body is traced R times; (t(R=r_hi) - t(R=1)) / (r_hi - 1)
    cancels the per-execution runtime/RPC overhead (~0.5 ms in this
    container) that every NEFF pays regardless of its contents.
    Benches run serially per core (quiet machine) with interleaved
    low/high rounds; min of each is used."""
    import gc
    import time
    import jax

    vs, ws, thick = _host_strokes(**inputs)
    core_work = _plan_all(vs, ws, thick)
    devices = jax.devices()[:N_CORES]

    def bench(run):
        run()
        window = []  # bound in-flight execs (unbounded piles up output bufs)
        t0 = time.time()
        for _ in range(repeats - 1):
            window.append(run(block=False))
            if len(window) >= 12:
                o = window.pop(0)
                for v in o.values():
                    v.block_until_ready()
        run(block=True)
        return (time.time() - t0) / repeats

    times = []
    for c in cores if cores is not None else range(N_CORES):
        nc1, in_map1, _ = _build_core_program(core_work[c], repeat=1)
        run1 = _make_exec(nc1, in_map1, devices[c])
        nch, in_maph, _ = _build_core_program(core_work[c], repeat=r_hi)
        runh = _make_exec(nch, in_maph, devices[c])
        run1()
        runh()
        t1s, ths = [], []
        for _ in range(rounds):
            t1s.append(bench(run1))
            ths.append(bench(runh))
        t1, th = min(t1s), min(ths)
        times.append(max(0.0, (th - t1) / (r_hi - 1)))
        del run1, runh, nc1, nch
        gc.collect()
    return times



# revision 2
# speedup vs baseline: 5.7266x; 5.7266x over previous
"""Trainium2 Bass kernel for nn_BezierRenderer (v2).

out[b] = max over 10 segments of clip((thick_b - dist(pixel, seg)) / thick_b, 0, 1)

Pipeline (per core, per 512-column PSUM chunk; columns = packed per-segment
bounding-box windows from many strokes/tiles, block-diagonal K rows):

  PE   mm_z:  Z = z~ plane scaled 1/thick (affine in (p^,f); bf16 hi/lo splits)
  ACT  a = Abs(Z)                 PSUM -> SBUF fp16
  Pool u = a - h_plane            (fp16, all SBUF; h_plane = h/thick, fp16)
  DVE  e2 = max(0,u)*u -> PSUM D  (relu^2, scalar_tensor_tensor)
  PE   mm_w: w^2/thick^2 quadratic plane accumulated onto D (start=False)
             (+eps row keeps D >= 0; validated host-side per segment)
  ACT  S = Sqrt(D) -> SBUF bf16   (= dist/thick)
  Pool T = ones - S               (= 1 - dist/thick)
  DVE  acc[tile window] = max(acc, T)   per segment (tt max, bf16)

acc starts at 0 and is exactly the darkness plane -> DMA out, host scatters.

Work is split stroke-tile-wise across the 8 NeuronCores (greedy balance);
each core runs its own specialized Bass program via PJRT device pinning.
"""

import threading
from contextlib import ExitStack

import numpy as np
import ml_dtypes

BF16 = ml_dtypes.bfloat16

# ---------------------------------------------------------------------------
# problem constants (hardcoded; kernel.py must be self-contained)
# ---------------------------------------------------------------------------
SIZE = 512
NUM_CTRL = 4
P = 10
B = 16
N_CORES = 8
MARGIN_PAD = 1.5
CHUNK_W = 512  # PSUM bank: 512 fp32 cols

# planner cost model (ns-ish units)
C_COL = 4.6      # per packed column (sum of per-engine per-col costs / overlap)
C_SEG = 190.0    # per segment (max-acc instr + misc)
C_TILE = 700.0   # per tile (memset + out-DMA + stationary rows)


def bf(x):
    return np.asarray(x).astype(BF16)


def split2(x):
    """x -> (hi, lo) bf16 rows whose fp32 sum ~= x."""
    hi = np.asarray(x, np.float64)
    h1 = bf(hi).astype(np.float64)
    l1 = bf(hi - h1).astype(np.float64)
    return h1, l1


def split3(x):
    h1 = bf(x).astype(np.float64)
    r = np.asarray(x, np.float64) - h1
    h2 = bf(r).astype(np.float64)
    h3 = bf(r - h2).astype(np.float64)
    return h1, h2, h3


# ---------------------------------------------------------------------------
# host-side geometry (mirrors reference.py numerics)
# ---------------------------------------------------------------------------
def _bezier_weights():
    M = 2 * P
    n = np.arange(M) - (M - 1) / 2.0
    gaus = np.exp(-0.5 * (n / 2.0) ** 2) * 0.75
    W = np.zeros((NUM_CTRL, P), dtype=np.float32)
    for i in range(NUM_CTRL):
        start = int(P - P * (i / (NUM_CTRL - 1)))
        W[i, :] = gaus[start : start + P]
    return W


def _host_strokes(trajectories, thicknesses):
    W = _bezier_weights()
    traj = np.asarray(trajectories, dtype=np.float32)
    sample = np.einsum("bck,kp->bpc", traj, W).astype(np.float32)
    last = traj[:, :, 3][:, None, :]
    stroke = np.concatenate([sample, last], axis=1).astype(np.float32)
    stroke = stroke * np.float32(SIZE)  # (B, P+1, 2) [y, x]
    vs = stroke[:, :-1]
    ws = stroke[:, 1:]
    th = np.asarray(thicknesses, dtype=np.float32)[:, 0] * np.float32(2.0) + np.float32(0.5)
    thick = np.float32(2.0) * th.sum(-1, dtype=np.float32)  # (B,)
    return vs, ws, thick


# ---------------------------------------------------------------------------
# planning
# ---------------------------------------------------------------------------
class Seg:
    __slots__ = ("w_lo", "w_hi", "vp", "vf", "wp", "wf", "h")

    def __init__(self, w_lo, w_hi, vp, vf, wp, wf):
        self.w_lo = w_lo
        self.w_hi = w_hi
        self.vp = vp
        self.vf = vf
        self.wp = wp
        self.wf = wf


class Tile:
    __slots__ = ("stroke", "transposed", "p_lo", "p_ext", "f_lo", "f_ext",
                 "thick", "segs")

    def __init__(self, stroke, transposed, p_lo, p_ext, thick):
        self.stroke = stroke
        self.transposed = transposed
        self.p_lo = p_lo
        self.p_ext = p_ext
        self.thick = thick
        self.segs = []
        self.f_lo = 0
        self.f_ext = 0


def _plan_stroke_orient(b, v, w, thick, transposed):
    """Plan tiles+segments for one stroke at a given orientation.
    Returns (tiles, cost)."""
    margin = float(thick) + MARGIN_PAD
    PAX, FAX = (1, 0) if transposed else (0, 1)
    lo = np.minimum(v, w).min(axis=0) - margin
    hi = np.maximum(v, w).max(axis=0) + margin
    plo = max(0, int(np.floor(lo[PAX])))
    phi = min(SIZE, int(np.ceil(hi[PAX])) + 1)
    if phi <= plo:
        return [], 0.0

    tiles = []
    cost = 0.0
    n_pb = (phi - plo + 127) // 128
    for pb in range(n_pb):
        p_lo = plo + pb * 128
        p_ext = min(128, phi - p_lo)
        tile = Tile(b, transposed, p_lo, p_ext, thick)
        for s in range(P):
            vp, vf = v[s][PAX], v[s][FAX]
            wp, wf = w[s][PAX], w[s][FAX]
            blo, bhi = p_lo - margin, p_lo + p_ext - 1 + margin
            if abs(wp - vp) < 1e-12:
                if vp < blo or vp > bhi:
                    continue
                t0, t1 = 0.0, 1.0
            else:
                ta = (blo - vp) / (wp - vp)
                tb = (bhi - vp) / (wp - vp)
                t0, t1 = max(0.0, min(ta, tb)), min(1.0, max(ta, tb))
                if t1 < t0:
                    continue
            fa = vf + t0 * (wf - vf)
            fb = vf + t1 * (wf - vf)
            w_lo = max(0, int(np.floor(min(fa, fb) - margin)))
            w_hi = min(SIZE, int(np.ceil(max(fa, fb) + margin)) + 1)
            if w_hi <= w_lo:
                continue
            tile.segs.append(Seg(w_lo, w_hi, vp, vf, wp, wf))
            cost += C_SEG + C_COL * (w_hi - w_lo)
        if tile.segs:
            f_lo = min(s.w_lo for s in tile.segs) & ~1
            f_hi = min(SIZE, (max(s.w_hi for s in tile.segs) + 1) & ~1)
            tile.f_lo, tile.f_ext = f_lo, f_hi - f_lo
            tiles.append(tile)
            cost += C_TILE + 0.4 * tile.f_ext
    return tiles, cost


def _plan_all(vs, ws, thick):
    """Choose orientation per stroke, then greedily balance tiles across
    cores. Returns core_tiles: list (per core) of Tile."""
    units = []
    for b in range(B):
        v = vs[b].astype(np.float64)
        w = ws[b].astype(np.float64)
        best = None
        for tr in (False, True):
            tiles, cost = _plan_stroke_orient(b, v, w, float(thick[b]), tr)
            if best is None or cost < best[1]:
                best = (tiles, cost)
        for t in best[0]:
            tcost = C_TILE + 0.4 * t.f_ext + sum(
                C_SEG + C_COL * (s.w_hi - s.w_lo) for s in t.segs)
            units.append((tcost, t))
    units.sort(key=lambda u: u[0], reverse=True)
    core_cost = [0.0] * N_CORES
    core_tiles = [[] for _ in range(N_CORES)]
    for tcost, t in units:
        c = min(range(N_CORES), key=lambda i: core_cost[i])
        core_cost[c] += tcost
        core_tiles[c].append(t)
    return core_tiles


# ---------------------------------------------------------------------------
# per-core program construction
# ---------------------------------------------------------------------------
def _seg_rows(tile, seg):
    """Plane coefficient rows for one segment, scaled 1/thick.

    Returns dict with:
      zA(f) fp64 array over window, zB scalar   (z''-plane = zA + zB*p^)
      h16 fp16 scalar
      wC2(f), wB2(f) arrays, wA2 scalar         (w-plane quad, +eps applied later)
    """
    th = tile.thick
    vp, vf, wp, wf = seg.vp, seg.vf, seg.wp, seg.wf
    dp, df = wp - vp, wf - vf
    d2 = dp * dp + df * df
    f = np.arange(seg.w_lo, seg.w_hi, dtype=np.float64)
    r_c = (tile.p_ext - 1) / 2.0
    P_c = tile.p_lo + r_c
    if d2 > 1e-4:
        d2p = d2 + 1e-5
        m = np.sqrt(d2p)
        h = m / 2.0
        hp = h / th
        h16 = np.float16(hp)
        gam = float(h16) / hp  # fold fp16(h') error into z scale
        # z~ = ((p-vp)dp + (f-vf)df)/m - h ; z'' = gam * z~ / th
        sz = gam / (th * m)
        zA = ((P_c - vp) * dp + (f - vf) * df) * sz - gam * hp
        zB = dp * sz
        # w = ((p-vp)df - (f-vf)dp)/sqrt(d2) ; w' = w/th
        sw = 1.0 / (th * np.sqrt(d2))
        C = ((P_c - vp) * df - (f - vf) * dp) * sw
        E = df * sw
        wC2 = C * C
        wB2 = 2.0 * E * C
        wA2 = E * E
    else:
        # degenerate: point distance to v
        h16 = np.float16(1.0)
        zA = np.zeros_like(f)
        zB = 0.0
        # w'^2 = ((p-vp)^2 + (f-vf)^2)/th^2
        it = 1.0 / th
        C = (f - vf) * it       # f-part
        Cp = (P_c - vp) * it    # p-part const
        Ep = it
        wC2 = C * C + Cp * Cp
        wB2 = 2.0 * Ep * Cp + 0.0 * f
        wA2 = Ep * Ep
    return dict(zA=zA, zB=float(zB), h16=h16, wC2=wC2, wB2=wB2, wA2=float(wA2))


class Chunk:
    __slots__ = ("entries", "cols", "t0", "t1")

    def __init__(self):
        self.entries = []  # (tile_idx, seg, col_off)
        self.cols = 0
        self.t0 = None  # first tile idx
        self.t1 = None  # last tile idx + 1


KZT, KWT = 4, 11  # stationary rows per tile (z-side, w-side)


def _build_core_program(tiles, repeat=1):
    import concourse.bass as bass
    import concourse.mybir as mybir
    import concourse.tile as tile_mod

    n_tiles = max(1, len(tiles))
    assert KWT * n_tiles <= 128, "too many tiles on one core"

    # ---- chunk packing (segments in tile order; chunk sizes ramp up) ----
    sizes = [128, 256] + [CHUNK_W] * 64
    chunks = []
    cur = Chunk()
    cap = sizes[0]
    for ti, t in enumerate(tiles):
        for seg in t.segs:
            fw = seg.w_hi - seg.w_lo
            if cur.cols + fw > cap:
                if cur.entries:
                    chunks.append(cur)
                cur = Chunk()
                cap = sizes[len(chunks)]
            if cur.t0 is None:
                cur.t0 = ti
            cur.t1 = ti + 1
            cur.entries.append((ti, seg, cur.cols))
            cur.cols += fw
    if cur.entries:
        chunks.append(cur)

    phat = np.arange(128, dtype=np.float64)

    # ---- global stationaries: z rows at 4*ti (tensor A), w rows at 11*ti (B)
    stat_z = np.zeros((128, 128), np.float64)
    stat_w = np.zeros((128, 128), np.float64)
    for ti, t in enumerate(tiles):
        r_c = (t.p_ext - 1) / 2.0
        ph = phat - r_c
        p2 = ph * ph
        p2h = bf(p2).astype(np.float64)
        p2l = p2 - p2h
        rz = KZT * ti
        stat_z[rz + 0] = 1.0
        stat_z[rz + 1] = 1.0
        stat_z[rz + 2] = ph
        stat_z[rz + 3] = ph
        rw = KWT * ti
        stat_w[rw + 0] = 1.0
        stat_w[rw + 1] = 1.0
        stat_w[rw + 2] = 1.0
        stat_w[rw + 3] = ph
        stat_w[rw + 4] = ph
        stat_w[rw + 5] = ph
        stat_w[rw + 6] = p2h
        stat_w[rw + 7] = p2h
        stat_w[rw + 8] = p2h
        stat_w[rw + 9] = p2l
        stat_w[rw + 10] = p2l

    # ---- per-chunk packed consts:
    #   packA [128, 2W]: cols [0:W) h plane (fp16 bits), [W:2W) z-rhs rows
    #   packB [128, W]:  w-rhs rows
    packAs, packBs = [], []
    for ch in chunks:
        W = ch.cols
        pa = np.zeros((128, 2 * W), np.uint16)
        pb = np.zeros((128, W), np.uint16)
        h_cols = np.zeros(W, np.float16)
        for ti, seg, off in ch.entries:
            t = tiles[ti]
            fw = seg.w_hi - seg.w_lo
            g = _seg_rows(t, seg)
            sl = slice(off, off + fw)
            h_cols[sl] = g["h16"]
            r_c = (t.p_ext - 1) / 2.0
            ph = phat - r_c
            p2 = ph * ph
            p2h = bf(p2).astype(np.float64)
            p2l_b = bf(p2 - p2h).astype(np.float64)
            zAh, zAl = split2(g["zA"])
            zBh, zBl = split2(g["zB"])
            rowsA = np.zeros((128, fw), np.float64)
            rz = KZT * ti
            rowsA[rz + 0] = zAh
            rowsA[rz + 1] = zAl
            rowsA[rz + 2] = zBh
            rowsA[rz + 3] = zBl
            B2a, B2b, B2c = split3(g["wB2"])
            A2a, A2b, A2c = split3(g["wA2"])
            C2a, C2b, C2c = split3(g["wC2"])
            pl = (C2a + C2b + C2c)[None, :] \
                + ph[:, None] * (B2a + B2b + B2c)[None, :] \
                + (p2h * (A2a + A2b + A2c) + p2l_b * (A2a + A2b))[:, None]
            mn = pl.min()
            pl_abs = (np.abs(C2a) + np.abs(C2b) + np.abs(C2c))[None, :] \
                + np.abs(ph)[:, None] * (np.abs(B2a) + np.abs(B2b) + np.abs(B2c))[None, :] \
                + (p2h * (np.abs(A2a) + np.abs(A2b) + np.abs(A2c))
                   + np.abs(p2l_b) * (np.abs(A2a) + np.abs(A2b)))[:, None]
            eps = max(0.0, -float(mn)) * 1.3 + float(pl_abs.max()) * 1.2e-7 + 1e-7
            C2a, C2b, C2c = split3(g["wC2"] + eps)
            rowsB = np.zeros((128, fw), np.float64)
            rw = KWT * ti
            rowsB[rw + 0] = C2a
            rowsB[rw + 1] = C2b
            rowsB[rw + 2] = C2c
            rowsB[rw + 3] = B2a
            rowsB[rw + 4] = B2b
            rowsB[rw + 5] = B2c
            rowsB[rw + 6] = A2a
            rowsB[rw + 7] = A2b
            rowsB[rw + 8] = A2c
            rowsB[rw + 9] = A2a
            rowsB[rw + 10] = A2b
            pa[:, W + off:W + off + fw] = bf(rowsA).view(np.uint16)
            pb[:, off:off + fw] = bf(rowsB).view(np.uint16)
        pa[:, :W] = np.broadcast_to(h_cols.view(np.uint16), (128, W))
        packAs.append(pa.view(BF16))
        packBs.append(pb.view(BF16))

    # acc layout: tile ti -> cols [acc_off[ti], acc_off[ti]+f_ext)
    acc_off = []
    o = 0
    for t in tiles:
        acc_off.append(o)
        o += t.f_ext
    acc_cols = max(2, o)

    # ---- trace program ----
    nc = bass.Bass()
    in_map = {"statz": bf(stat_z), "statw": bf(stat_w)}
    statz_e = nc.dram_tensor("statz", [128, 128], mybir.dt.bfloat16,
                             kind="ExternalInput")
    statw_e = nc.dram_tensor("statw", [128, 128], mybir.dt.bfloat16,
                             kind="ExternalInput")
    pa_e, pb_e = [], []
    for ci in range(len(chunks)):
        nmA, nmB = f"packA{ci}", f"packB{ci}"
        pa_e.append(nc.dram_tensor(nmA, list(packAs[ci].shape),
                                   mybir.dt.bfloat16, kind="ExternalInput"))
        pb_e.append(nc.dram_tensor(nmB, list(packBs[ci].shape),
                                   mybir.dt.bfloat16, kind="ExternalInput"))
        in_map[nmA] = packAs[ci]
        in_map[nmB] = packBs[ci]
    out_ext = nc.dram_tensor("out", [128, acc_cols], mybir.dt.bfloat16,
                             kind="ExternalOutput")

    with tile_mod.TileContext(nc) as tc:
        with ExitStack() as ctx:
            const = ctx.enter_context(tc.tile_pool(name="const", bufs=1))
            accp = ctx.enter_context(tc.tile_pool(name="accp", bufs=1))
            sb = ctx.enter_context(tc.tile_pool(name="work", bufs=4))
            psum = ctx.enter_context(tc.tile_pool(name="psum", bufs=4, space="PSUM"))

            t_statz = const.tile([128, 128], mybir.dt.bfloat16, tag="statz")
            nc.sync.dma_start(t_statz[:], statz_e[:])
            t_statw = const.tile([128, 128], mybir.dt.bfloat16, tag="statw")
            nc.sync.dma_start(t_statw[:], statw_e[:])
            t_pa, t_pb = [], []
            for ci in range(len(chunks)):
                ta = const.tile(list(packAs[ci].shape), mybir.dt.bfloat16,
                                tag=f"packA{ci}")
                tb = const.tile(list(packBs[ci].shape), mybir.dt.bfloat16,
                                tag=f"packB{ci}")
                engA = nc.sync if ci % 2 == 0 else nc.gpsimd
                engB = nc.gpsimd if ci % 2 == 0 else nc.sync
                engA.dma_start(ta[:], pa_e[ci][:])
                engB.dma_start(tb[:], pb_e[ci][:])
                t_pa.append(ta)
                t_pb.append(tb)
            t_ones = const.tile([128, CHUNK_W], mybir.dt.bfloat16, tag="ones")
            nc.gpsimd.memset(t_ones[:], 1.0)

            for _rep in range(repeat):
                t_acc = accp.tile([128, acc_cols], mybir.dt.bfloat16, tag="acc")
                nc.gpsimd.memset(t_acc[:], 0.0)

                for ci, ch in enumerate(chunks):
                    W = ch.cols
                    kz = KZT * ch.t1
                    kw = KWT * ch.t1
                    h_ap = t_pa[ci][:, 0:W].bitcast(mybir.dt.float16)
                    zp = psum.tile([128, CHUNK_W], mybir.dt.float32, tag="zp")
                    nc.tensor.matmul(zp[:, :W], t_statz[:kz, :],
                                     t_pa[ci][:kz, W:2 * W],
                                     start=True, stop=True)
                    a_t = sb.tile([128, CHUNK_W], mybir.dt.float16, tag="a")
                    nc.scalar.activation(a_t[:, :W], zp[:, :W],
                                         mybir.ActivationFunctionType.Abs)
                    u_t = sb.tile([128, CHUNK_W], mybir.dt.float16, tag="u")
                    nc.gpsimd.tensor_tensor(u_t[:, :W], a_t[:, :W], h_ap,
                                            mybir.AluOpType.subtract)
                    dp = psum.tile([128, CHUNK_W], mybir.dt.float32, tag="dp")
                    nc.vector.scalar_tensor_tensor(
                        dp[:, :W], u_t[:, :W], 0.0, u_t[:, :W],
                        mybir.AluOpType.max, mybir.AluOpType.mult)
                    nc.tensor.matmul(dp[:, :W], t_statw[:kw, :],
                                     t_pb[ci][:kw, :W],
                                     start=False, stop=True, skip_group_check=True)
                    s_t = sb.tile([128, CHUNK_W], mybir.dt.bfloat16, tag="s")
                    nc.scalar.activation(s_t[:, :W], dp[:, :W],
                                         mybir.ActivationFunctionType.Sqrt)
                    T_t = sb.tile([128, CHUNK_W], mybir.dt.bfloat16, tag="T")
                    nc.gpsimd.tensor_tensor(T_t[:, :W], t_ones[:, :W], s_t[:, :W],
                                            mybir.AluOpType.subtract)
                    for ti, seg, off in ch.entries:
                        t = tiles[ti]
                        fw = seg.w_hi - seg.w_lo
                        c0 = acc_off[ti] + seg.w_lo - t.f_lo
                        dst = t_acc[:t.p_ext, c0:c0 + fw]
                        nc.vector.tensor_tensor(dst, dst,
                                                T_t[:t.p_ext, off:off + fw],
                                                mybir.AluOpType.max)

                nc.sync.dma_start(out_ext[:, :acc_cols], t_acc[:, :acc_cols])

    _split_multiwait(nc, mybir)
    meta = (tiles, acc_off)
    return nc, in_map, meta


# ---------------------------------------------------------------------------
# walrus compat: at most one semaphore wait per instruction
# ---------------------------------------------------------------------------
def _split_multiwait(nc, mybir):
    for fn in nc.m.functions:
        for bb in fn.blocks:
            insts = bb.instructions
            idx = 0
            while idx < len(insts):
                inst = insts[idx]
                si = inst.sync_info
                ow = list(si.on_wait) if (si and si.on_wait) else []
                if len(ow) > 1:
                    si.on_wait = ow[-1:]
                    for j, w in enumerate(ow[:-1]):
                        nop = mybir.InstNoOp(
                            name=f"{inst.name}-ws{j}",
                            engine=inst.engine,
                            ins=[],
                            outs=[],
                            sync_info=mybir.SyncInfo(on_wait=[w], on_update=[]),
                        )
                        nc.register_instruction(nop, overwrite=True)
                        insts.insert(idx, nop)
                        idx += 1
                idx += 1


# ---------------------------------------------------------------------------
# MPMD runner (one program per core, pinned via jax.default_device)
# ---------------------------------------------------------------------------
def _make_exec(nc, in_map, device):
    import jax
    import concourse.mybir as mybir
    from concourse import bass2jax

    bass2jax.install_neuronx_cc_hook()
    partition_name = nc.partition_id_tensor.name if nc.partition_id_tensor else None
    in_names, out_names, out_avals, zero_shapes = [], [], [], []
    for alloc in nc.m.functions[0].allocations:
        if not isinstance(alloc, mybir.MemoryLocationSet):
            continue
        name = alloc.memorylocations[0].name
        if alloc.kind == "ExternalInput":
            if name != partition_name:
                in_names.append(name)
        elif alloc.kind == "ExternalOutput":
            out_names.append(name)
            shape = tuple(alloc.tensor_shape)
            dtype = mybir.dt.np(alloc.dtype)
            out_avals.append(jax.core.ShapedArray(shape, dtype))
            zero_shapes.append((shape, dtype))
    n_params = len(in_names)
    all_in_names = list(in_names) + out_names
    if partition_name is not None:
        all_in_names.append(partition_name)
    donate = tuple(range(n_params, n_params + len(out_names)))

    def _body(*args):
        operands = list(args)
        if partition_name is not None:
            operands.append(bass2jax.partition_id_tensor())
        outs = bass2jax._bass_exec_p.bind(
            *operands,
            out_avals=tuple(out_avals),
            in_names=tuple(all_in_names),
            out_names=tuple(out_names),
            lowering_input_output_aliases=(),
            sim_require_finite=False,
            sim_require_nnan=False,
            nc=nc,
        )
        return tuple(outs)

    fn = jax.jit(_body, donate_argnums=donate, keep_unused=True)
    args = [np.asarray(in_map[n]) for n in in_names]

    def run(block=True):
        with jax.default_device(device):
            outs = fn(*args, *[np.zeros(s, d) for s, d in zero_shapes])
        if block:
            for o in outs:
                o.block_until_ready()
        return {name: outs[i] for i, name in enumerate(out_names)}

    return run


_CACHE = {}


def _prepare(trajectories, thicknesses):
    import jax

    key = (np.asarray(trajectories).tobytes(), np.asarray(thicknesses).tobytes())
    if key in _CACHE:
        return _CACHE[key]
    vs, ws, thick = _host_strokes(trajectories, thicknesses)
    core_tiles = _plan_all(vs, ws, thick)
    progs = [_build_core_program(core_tiles[c]) for c in range(N_CORES)]
    devices = jax.devices()[:N_CORES]
    runners = [None] * N_CORES
    errors = []

    def make(c):
        try:
            nc, in_map, _ = progs[c]
            runners[c] = _make_exec(nc, in_map, devices[c])
            runners[c]()
        except Exception as e:  # pragma: no cover
            errors.append((c, e))

    threads = [threading.Thread(target=make, args=(c,)) for c in range(N_CORES)]
    for t in threads:
        t.start()
    for t in threads:
        t.join()
    if errors:
        raise errors[0][1]
    _CACHE[key] = (progs, runners)
    return _CACHE[key]


def kernel(trajectories, thicknesses):
    trajectories = np.asarray(trajectories)
    thicknesses = np.asarray(thicknesses)
    progs, runners = _prepare(trajectories, thicknesses)

    results = [None] * N_CORES
    errors = []

    def runner(c):
        try:
            results[c] = runners[c]()
        except Exception as e:  # pragma: no cover
            errors.append((c, e))

    threads = [threading.Thread(target=runner, args=(c,)) for c in range(N_CORES)]
    for t in threads:
        t.start()
    for t in threads:
        t.join()
    if errors:
        raise errors[0][1]

    canvas = np.zeros((B, SIZE, SIZE), dtype=np.float32)
    for c in range(N_CORES):
        _, _, (tiles, acc_off) = progs[c]
        out = np.asarray(results[c]["out"]).astype(np.float32)
        for ti, t in enumerate(tiles):
            block = out[:t.p_ext, acc_off[ti]:acc_off[ti] + t.f_ext]
            if t.transposed:
                canvas[t.stroke, t.f_lo:t.f_lo + t.f_ext,
                       t.p_lo:t.p_lo + t.p_ext] = block.T
            else:
                canvas[t.stroke, t.p_lo:t.p_lo + t.p_ext,
                       t.f_lo:t.f_lo + t.f_ext] = block
    np.maximum(canvas, 0.0, out=canvas)
    return canvas


def time_cores(inputs, repeats=400, r_hi=9, rounds=3, cores=None):
    """Differential per-core device time: (t(R=r_hi)-t(R=1))/(r_hi-1)."""
    import gc
    import time
    import jax

    vs, ws, thick = _host_strokes(**inputs)
    core_tiles = _plan_all(vs, ws, thick)
    devices = jax.devices()[:N_CORES]

    def bench(run):
        run()
        window = []
        t0 = time.time()
        for _ in range(repeats - 1):
            window.append(run(block=False))
            if len(window) >= 12:
                o = window.pop(0)
                for v in o.values():
                    v.block_until_ready()
        run(block=True)
        return (time.time() - t0) / repeats

    times = []
    for c in cores if cores is not None else range(N_CORES):
        nc1, im1, _ = _build_core_program(core_tiles[c], repeat=1)
        run1 = _make_exec(nc1, im1, devices[c])
        nch, imh, _ = _build_core_program(core_tiles[c], repeat=r_hi)
        runh = _make_exec(nch, imh, devices[c])
        run1()
        runh()
        t1s, ths = [], []
        for _ in range(rounds):
            t1s.append(bench(run1))
            ths.append(bench(runh))
        t1, th = min(t1s), min(ths)
        times.append(max(0.0, (th - t1) / (r_hi - 1)))
        del run1, runh, nc1, nch
        gc.collect()
    return times
